# revision 4
# baseline (speedup 1.0000x reference)
"""LocalMeanInpainter Trainium2 kernel, v5.

out = x*mask + (box15(x)/box15(ones))*(1-mask)  over (32,3,512,512) f32.

v4 = v1 (banded bf16 separable matmuls, per-chunk out DMAs) with the
device OUTPUT in bf16 (host upcasts to f32): kept pixels are bf16(x)
exactly as before (x is shipped bf16), the box-mean path gains only
~1e-3 relative error — far inside the 2e-2 gate — and output HBM
traffic halves. Per-core HBM: x 6.29MB + mask u8 3.15MB + B 0.14MB in,
out bf16 6.29MB -> 15.9MB (~50us at the measured ~320GB/s/core).
Also: band-compressed B (144 cols/chunk) and deeper tile pools.
"""

import numpy as np
import ml_dtypes

H = 512
W = 512
WINDOW = 15
PAD = 7
N_CORES = 8
IMGS_PER_CORE = 4
CHANNELS = 3
PLANES = IMGS_PER_CORE * CHANNELS  # 12
NCHUNK = H // 128  # 4
BBAND = 144  # padded band width per chunk (max real: 142)

_BS = [max(128 * k - PAD, 0) for k in range(NCHUNK)]
_BE = [min(128 * k + 128 + PAD, 512) for k in range(NCHUNK)]

# engine for the blend "fill ot with x" copy, per 128-chunk
# (bf16->bf16 SBUF copies hit the DVE 4x mode, so DVE is cheap here)
B_ENGINES = ("dve", "dve", "dve", "dve")
# engine for the ps1 -> s1b copy (class A), per chunk
A_ENGINES = ("act", "act", "act", "act")

_CACHE = {}


def _band_matrix(n):
    idx = np.arange(n)
    band = (np.abs(idx[:, None] - idx[None, :]) <= PAD).astype(np.float64)
    cnt = np.minimum(idx + PAD, n - 1) - np.maximum(idx - PAD, 0) + 1
    return band / cnt[None, :]


def _build_program(planes=PLANES, reps=1, ablate=()):
    import concourse.tile as tile
    from concourse import bacc, mybir
    from contextlib import nullcontext

    f32 = mybir.dt.float32
    bf16 = mybir.dt.bfloat16
    u8 = mybir.dt.uint8

    nc = bacc.Bacc("TRN2", target_bir_lowering=False, debug=False, num_devices=N_CORES)
    x_d = nc.declare_dram_parameter("x", [planes, 128, NCHUNK * W], bf16, isOutput=False)
    m_d = nc.declare_dram_parameter("maskinv", [planes, 128, NCHUNK * W], u8, isOutput=False)
    b_d = nc.declare_dram_parameter("b", [128, NCHUNK * BBAND], bf16, isOutput=False)
    out_d = nc.declare_dram_parameter("out", [planes, 128, NCHUNK * W], bf16, isOutput=True)

    with tile.TileContext(nc) as tc:
        with (
            tc.tile_pool(name="consts", bufs=1) as cpool,
            tc.tile_pool(name="xt", bufs=4) as xpool,
            tc.tile_pool(name="mt", bufs=4) as mpool,
            tc.tile_pool(name="s1b", bufs=3) as s1pool,
            tc.tile_pool(name="ot", bufs=3) as opool,
            tc.tile_pool(name="ps1", bufs=3, space="PSUM") as ps1pool,
            tc.tile_pool(name="ps2", bufs=4, space="PSUM") as ps2pool,
        ):
            # band-compressed B: [128, (chunk, 144)], shared by both passes
            b_t = cpool.tile([128, NCHUNK * BBAND], bf16, tag="b")
            nc.sync.dma_start(out=b_t[:], in_=b_d[:])

            def mms(ps, lhsT_of):
                # Banded: chunk kc only touches output cols [128k-7, 128k+135).
                for kc in range(NCHUNK):
                    lo, hi = 128 * kc, 128 * (kc + 1)
                    segs = []
                    if kc > 0:
                        segs.append((lo - PAD, lo + PAD, False, True))
                    e0 = lo if kc == 0 else lo + PAD
                    e1 = hi if kc == NCHUNK - 1 else hi - PAD
                    segs.append((e0, e1, True, True))
                    if kc < NCHUNK - 1:
                        segs.append((hi - PAD, hi + PAD, True, False))
                    lhsT = lhsT_of(kc)
                    for c0, c1, st, sp in segs:
                        boff = kc * BBAND + (c0 - _BS[kc])
                        nc.tensor.matmul(
                            ps[:, c0:c1],
                            lhsT=lhsT,
                            rhs=b_t[:, boff : boff + (c1 - c0)],
                            start=st,
                            stop=sp,
                        )

            loop_ctx = (
                tc.For_i(
                    0,
                    reps,
                    1,
                    hint_engines=tuple(
                        getattr(mybir.EngineType, e)
                        for e in ("PE", "Activation", "DVE", "SP", "Pool")
                    ),
                )
                if reps > 1
                else nullcontext()
            )
            with loop_ctx:
              for p in range(planes):
                xt = xpool.tile([128, NCHUNK * W], bf16, tag="xt")
                if "no_in_dma" not in ablate:
                    nc.sync.dma_start(out=xt[:], in_=x_d[p])
                mt = mpool.tile([128, NCHUNK * W], u8, tag="mt")
                if "no_in_dma" not in ablate:
                    nc.sync.dma_start(out=mt[:], in_=m_d[p])

                # pass 1: S1T[wc] [128 w, 512 h_out] over h chunks
                s1b = s1pool.tile([128, NCHUNK * H], bf16, tag="s1b")
                for wc in ([] if "no_pe" in ablate else range(NCHUNK)):
                    ps1 = ps1pool.tile([128, H], f32, tag="ps1")
                    mms(
                        ps1,
                        lambda kc: xt[:, kc * W + wc * 128 : kc * W + wc * 128 + 128],
                    )
                    if A_ENGINES[wc] == "act":
                        nc.scalar.copy(s1b[:, wc * H : (wc + 1) * H], ps1[:])
                    else:
                        nc.vector.tensor_copy(s1b[:, wc * H : (wc + 1) * H], ps1[:])

                # pass 2 + blend (bf16 out) + per-chunk DMA out
                ot = opool.tile([128, NCHUNK * W], bf16, tag="ot")
                for mc in range(NCHUNK):
                    otm = ot[:, mc * W : (mc + 1) * W]
                    xtm = xt[:, mc * W : (mc + 1) * W]
                    mtm = mt[:, mc * W : (mc + 1) * W]
                    if "no_pe" in ablate:
                        nc.vector.tensor_copy(otm, xtm)
                    else:
                        ps2 = ps2pool.tile([128, W], f32, tag="ps2")
                        mms(
                            ps2,
                            lambda kc: s1b[
                                :, kc * H + mc * 128 : kc * H + mc * 128 + 128
                            ],
                        )
                        # fill with kept pixels (bf16 copy, DVE 4x mode) ...
                        if B_ENGINES[mc] == "act":
                            nc.scalar.copy(otm, xtm)
                        else:
                            nc.vector.tensor_copy(otm, xtm)
                        # ... and pull the box-mean (f32 PSUM -> bf16) where missing
                        nc.vector.copy_predicated(otm, mtm, ps2[:])
                op_idx = 0 if "out_same" in ablate else p
                if "no_out_dma" not in ablate:
                    nc.sync.dma_start(out=out_d[op_idx], in_=ot[:])
    nc.finalize()
    return nc


def _host_weights():
    B = _band_matrix(H)
    bb = np.zeros((128, NCHUNK * BBAND), dtype=np.float64)
    for kc in range(NCHUNK):
        w = _BE[kc] - _BS[kc]
        bb[:, kc * BBAND : kc * BBAND + w] = B[kc * 128 : (kc + 1) * 128, _BS[kc] : _BE[kc]]
    return bb.astype(ml_dtypes.bfloat16)


def make_in_maps(x: np.ndarray, mask: np.ndarray):
    """x, mask: full (32,3,512,512) f32 -> per-core input dicts."""
    if "b" not in _CACHE:
        _CACHE["b"] = _host_weights()
    b = _CACHE["b"]
    xb = (
        np.ascontiguousarray(x, dtype=np.float32)
        .astype(ml_dtypes.bfloat16)
        .reshape(N_CORES * PLANES, NCHUNK, 128, W)
        .transpose(0, 2, 1, 3)
    )
    minv = (
        (np.ascontiguousarray(mask) == 0)
        .astype(np.uint8)
        .reshape(N_CORES * PLANES, NCHUNK, 128, W)
        .transpose(0, 2, 1, 3)
    )
    xs = np.ascontiguousarray(xb).reshape(N_CORES, PLANES, 128, NCHUNK * W)
    ms = np.ascontiguousarray(minv).reshape(N_CORES, PLANES, 128, NCHUNK * W)
    return [{"x": xs[i], "maskinv": ms[i], "b": b} for i in range(N_CORES)]


def untranspose_out(out_cores: np.ndarray) -> np.ndarray:
    """device bf16 output [..., 128, (c w)] -> f32 full (32,3,512,512)."""
    o = np.asarray(out_cores).astype(np.float32)
    o = o.reshape(N_CORES * PLANES, 128, NCHUNK, W).transpose(0, 2, 1, 3)
    return np.ascontiguousarray(o).reshape(32, CHANNELS, H, W)


def _get_program():
    if "nc" not in _CACHE:
        _CACHE["nc"] = _build_program()
    return _CACHE["nc"]


def kernel(x: np.ndarray, mask: np.ndarray) -> np.ndarray:
    from concourse.bass_utils import run_bass_kernel_spmd

    nc = _get_program()
    in_maps = make_in_maps(x, mask)
    res = run_bass_kernel_spmd(nc, in_maps, core_ids=list(range(N_CORES)))
    out = np.stack([res.results[i]["out"] for i in range(N_CORES)])
    return untranspose_out(out)
